# revision 87
# baseline (speedup 1.0000x reference)
"""NonLocalBlock (spatial self-attention) Trainium2 Bass kernel.

Problem: x [4, 128, 64, 64]; 1x1 convs theta/phi/g -> softmax(theta^T phi) g
-> 1x1 conv out + residual.

Sharding (8 cores): core k -> (batch b = k//2, query-half h = k%2).
Each core holds the full keys/values for its batch (xkv [128, 4096], rolled
host-side so its 2048 queries are columns [0, 2048)).  Weights replicated.

Key structural ideas:

1. Fused value path, rank-127:  G = w_out @ w_g has sigma_128 ~ 1e-9, so
   G ~= C_out @ P_g with P_g = V^T[:127] and C_out = U[:, :127] * S[:127].
   The PV stationary chunks [m=128, 128] hold column 0 = ones and columns
   1..127 = (P_g x)^T, so a single PV matmul accumulates BOTH the attention
   value sum (rows 1..127) and the softmax denominator (row 0).  No
   dedicated denominator matmuls or reductions anywhere.  The ones channel
   itself is materialized by the ghat drain BIAS ([1,0,...,0]) -- zero
   extra instructions.

2. Host-side normalization:  out = C_out(y/den) + x + b == (C_out y)/den
   + x + b, so the device ships the *unnormalized* conv result (bf16; the
   0.4% relative quantization survives the division as ~4e-4) and den =
   row 0 of the value accumulator, DMA'd straight out of the yu cast.

3. Two-engine exp: ACT exp alternates strictly with a DVE Schraudolph
   bit-trick (ONE tensor_scalar op: i16 = round(s*128*log2(e) + B) whose
   int16 bits ARE bf16(exp(s))); the last 6 pairs split each exp across
   both engines so the stream tail drains fast.

4. p-state + DMA choreography: TRN2's PE HAM clock gate (1.2 -> 2.4 GHz)
   needs ~3.4-5.7us of gapless matmul activity, and it RE-throttles after
   any >3.4us idle window.  Warmup matmuls (const-tile 1-col first, then
   512-col on a zeroed scratch tile) bridge from the framework preamble to
   data arrival.  Inputs load via only the two HWDGE rings (ring startups
   serialize ~2us apiece in nondeterministic order): each priority piece
   (first-needed xkv columns, wt|wp weights) is split into two partition
   strips, one per ring, each in its own TILE so consumers wait only their
   own piece's DMAs.

5. One flat software pipeline over all 64 (block, key-pair) steps: QK runs
   PVD=3 pair-steps ahead of PV; the PV stream crosses block boundaries
   without draining.  ALL remaining projections (phi/ghat second halves,
   theta tail) are injected into the early pair stream -- drains placed
   opposite to that pair's exp engine -- and the first injected units
   borrow the still-idle attn/conv PSUM banks so the early QK pairs never
   stall on the 3-deep score-tile rotation.  Per-quarter ghat
   DMA-transposes land on alternating HWDGE queues just ahead of the PV
   stream (block-0 PVs gated until pg==5).

Steady state per pair: S^T [128m, 2, 512n] = phi^T theta (2 MMs), exp ->
pt bf16, attn_ps += ghatT^T pt (2 MMs); 216ns/512-col MM ~= the bf16 PE
roofline.  Block epilogue (yu cast, 1x1 conv, bf16 out + den DMAs) is
emitted inside the following block's stream; the last block's epilogue
runs as two pipelined half-chunks.
"""

import numpy as np

B, C = 4, 128
HW = 4096  # 64*64 spatial positions
QH = HW // 2  # queries per core
NCORES = 8
NBLK = 512  # query block size
NMCH = HW // 128  # 32 key chunks of 128
PVD = 3  # PV trails QK by this many pair-steps (= s_pool bufs)
WARMUP_MM = 17  # p-state ramp matmuls before the first projection
# pair indices handled by the DVE exp: strict alternation keeps either
# engine from falling a full exp behind the 3-deep PSUM rotation
DVE_PAIRS = {1, 3, 5, 7, 9, 11, 13}

# Schraudolph constants for bf16-via-int16: bitcast_bf16(round_i16(A*s + B))
EXP_A16 = 184.6649652337873  # 2^7 * log2(e)
EXP_B16 = 16250.409332        # 127*128 - 366392.7/65536

_CACHE = {}


def _legalize_waits(bir, verbose=False):
    """Split instructions carrying more sync waits than the gen3 ISA allows.

    Walrus caps sync waits at 1 per instruction (2 for EventSemaphore); the
    Tile tail drain and first-consumer instructions can exceed that. Spill
    excess waits onto inserted wait-only EventSemaphore instructions placed
    immediately before the offender on the same engine (engines execute
    in order, so this is semantics-preserving).
    """
    n_split = 0
    where = []
    for f in bir["functions"]:
        for bb in f["blocks"]:
            out = []
            for inst in bb["instructions"]:
                si = inst.get("sync_info")
                waits = (si or {}).get("on_wait") or []
                cap = 2 if inst["opcode"] == "EventSemaphore" else 1
                if len(waits) > cap:
                    excess = waits[:-cap]
                    si["on_wait"] = waits[-cap:]
                    for i in range(0, len(excess), 2):
                        chunk = excess[i : i + 2]
                        out.append(
                            {
                                "debug": inst.get("debug", 0),
                                "engine": inst["engine"],
                                "ins": [],
                                "name": f'{inst["name"]}_w{i}',
                                "opcode": "EventSemaphore",
                                "outs": [],
                                "sync_info": {"on_update": [], "on_wait": chunk},
                            }
                        )
                        n_split += 1
                    where.append((inst["name"], inst["opcode"], len(excess)))
                out.append(inst)
            bb["instructions"] = out
    if verbose and where:
        print(f"[legalize_waits] {n_split} wait insts inserted for:")
        for nm, op, ne in where:
            print(f"  {nm} ({op}): {ne} excess waits")
    return bir


def _build():
    from contextlib import ExitStack

    import concourse.bass as bass
    import concourse.tile as tile
    from concourse import mybir

    f32 = mybir.dt.float32
    bf16 = mybir.dt.bfloat16
    i16 = mybir.dt.int16

    Exp = mybir.ActivationFunctionType.Exp
    Copy = mybir.ActivationFunctionType.Copy

    nc = bass.Bass()
    # all big inputs pre-cast to bf16 host-side: halves DMA traffic and
    # lets the projections run as bf16 matmuls with no on-device casts
    x_kv = nc.dram_tensor("xkv", [C, HW], bf16, kind="ExternalInput")
    wts_d = nc.dram_tensor("wts", [C, 4 * C], bf16, kind="ExternalInput")
    bias_d = nc.dram_tensor("bias", [C, 3], f32, kind="ExternalInput")
    # out ships bf16: it is the UNNORMALIZED conv result (host divides by
    # den in f32), so bf16's 0.4% relative error survives the division as
    # ~4e-4 of the final output — and the tail-critical out DMA halves
    out_d = nc.dram_tensor("out", [C, QH], bf16, kind="ExternalOutput")
    # den ships as bf16 straight out of yu row 0 (the ones-channel PV
    # accumulator), so no dedicated DVE copy; 0.4% quantization on the
    # denominator is ~2e-4 relative on the final output
    den_d = nc.dram_tensor("den", [QH // NBLK, NBLK], bf16, kind="ExternalOutput")

    with ExitStack() as ctx:
        tc = ctx.enter_context(tile.TileContext(nc))
        const = ctx.enter_context(tc.tile_pool(name="const", bufs=1))
        persist = ctx.enter_context(tc.tile_pool(name="persist", bufs=1))
        small = ctx.enter_context(tc.tile_pool(name="small", bufs=2))
        pt_pool = ctx.enter_context(tc.tile_pool(name="pt", bufs=16))

        # ---- loads.  Two facts drive the layout: (1) only two HWDGE rings
        # exist and their ~2us startups serialize in nondeterministic
        # order, so the critical pieces must sit at the HEAD of BOTH rings;
        # (2) a consumer waits on every DMA that writes its TILE, so each
        # priority class gets its own tile.  xkv is cut into column pieces
        # (c0 = first-needed 1024 cols, c1, c2) and each piece into two
        # partition strips, one per ring, so a piece lands at
        # ring-start + its queue position regardless of which ring wins
        # the startup lottery. ----
        # scratch memset first on the gpsimd queue: it gates the p-state
        # warmup matmuls and gpsimd is free right after the framework
        # preamble
        scratch = const.tile([C, 512], bf16, tag="scratch")
        nc.gpsimd.memset(scratch, 0.0)
        xc = [
            persist.tile([C, n], bf16, tag=f"xc{j}", name=f"xc{j}")
            for j, n in enumerate((1024, 1024, 2048))
        ]
        bias_s = const.tile([C, 3], f32, tag="bias")
        wtp_s = const.tile([C, 2 * C], bf16, tag="wtp")  # wt | wp
        wpc_s = const.tile([C, 2 * C], bf16, tag="wpc")  # pg | co
        H2 = C // 2
        nc.scalar.dma_start(out=xc[0][0:H2, :], in_=x_kv[0:H2, 0:1024])
        nc.sync.dma_start(out=bias_s, in_=bias_d[:, :])
        nc.sync.dma_start(out=wtp_s, in_=wts_d[:, 0 : 2 * C])
        nc.scalar.dma_start(out=xc[1][0:H2, :], in_=x_kv[0:H2, 1024:2048])
        nc.sync.dma_start(out=xc[0][H2:, :], in_=x_kv[H2:, 0:1024])
        nc.scalar.dma_start(out=xc[2][0:H2, :], in_=x_kv[0:H2, 2048:4096])
        nc.sync.dma_start(out=xc[1][H2:, :], in_=x_kv[H2:, 1024:2048])
        nc.sync.dma_start(out=wpc_s, in_=wts_d[:, 2 * C :])
        nc.sync.dma_start(out=xc[2][H2:, :], in_=x_kv[H2:, 2048:4096])
        xkv_t = [
            (xc[0] if j < 2 else xc[1] if j < 4 else xc[2])[
                :, (j if j < 2 else j - 2 if j < 4 else j - 4) * 512 :][:, 0:512]
            for j in range(8)
        ]
        w_s = {
            "wt": wtp_s[:, 0:C],
            "wp": wtp_s[:, C:],
            "pg": wpc_s[:, 0:C],
            "co": wpc_s[:, C:],
        }
        # bg1 = [1, 0, ..., 0]: adding it in the ghat drains materializes the
        # ones channel (PV row 0 = softmax denominator) with no extra op --
        # the pg weight's k=0 column is zero, so row 0 = 0 + 1 = 1
        b_s = {"bt": bias_s[:, 0:1], "bp": bias_s[:, 1:2], "bg1": bias_s[:, 2:3]}

        # warm the ACT exp table while DMAs stream: the ACT_TABLE_LOAD walrus
        # emits before this instruction runs as soon as the scalar queue
        # reaches it (it does NOT wait for bias), so the one-time ~1.4us
        # table load is off the critical path by the time real exps start
        warm = const.tile([C, 1], f32, tag="warm")
        nc.scalar.activation(out=warm, in_=b_s["bt"], func=Exp, bias=0.0, scale=1.0)

        theta_s = persist.tile([C, QH], bf16, tag="theta")
        phi_t = [
            persist.tile([C, QH], bf16, tag=f"phi{t}", name=f"phi{t}")
            for t in range(2)
        ]
        gn_t = [
            persist.tile([C, QH], bf16, tag=f"gn{t}", name=f"gn{t}")
            for t in range(2)
        ]
        gT_t = [
            persist.tile([128, NMCH // 2, 128], bf16, tag=f"gT{t}", name=f"gT{t}")
            for t in range(2)
        ]

        # ---- projections (bf16 512-col matmuls; PSUM->SBUF drains split
        # between ACT and DVE so neither paces the PE stream).  One shared
        # PSUM pool serves warmup, projections AND the QK score tiles, so
        # the second-half projections can be injected INTO the early QK
        # stream of block 0 — the PE never idles long enough for the HAM
        # clock gate to re-throttle while waiting on the xkv second half
        # or the gT transposes. ----
        Ident = mybir.ActivationFunctionType.Identity
        s_pool = ctx.enter_context(tc.tile_pool(name="s_ps", bufs=PVD, space="PSUM"))
        attn_pool = ctx.enter_context(tc.tile_pool(name="attn_ps", bufs=1, space="PSUM"))
        conv_pool = ctx.enter_context(tc.tile_pool(name="conv_ps", bufs=1, space="PSUM"))

        warm_ps = s_pool.tile([128, 2, 512], f32, tag="s")
        # first ramp MMs use the framework const tile (written in the main
        # block, before the Tile body starts) so the HAM activity window
        # opens ~0.9us before the scratch memset even lands
        const1 = nc.const_aps.aps[(bf16, 1.0)]
        for _ in range(6):
            nc.tensor.matmul(warm_ps[0:1, 0, 0:1], const1, const1,
                             start=True, stop=True)
        for _ in range(WARMUP_MM):
            nc.tensor.matmul(warm_ps[:, 0, :], scratch[:, 0:128], scratch,
                             start=True, stop=True)

        def _drain(dst, ps, bias, act):
            if act == "pool":  # gpsimd drain: slow but otherwise idle
                nc.gpsimd.tensor_scalar_add(out=dst, in0=ps, scalar1=b_s[bias])
            elif act:  # ACT drain
                nc.scalar.activation(
                    out=dst,
                    in_=ps,
                    func=Ident,
                    bias=b_s[bias] if bias else 0.0,
                    scale=1.0,
                )
            elif bias is not None:  # DVE drain
                nc.vector.tensor_scalar_add(out=dst, in0=ps, scalar1=b_s[bias])
            else:
                nc.vector.tensor_copy(out=dst, in_=ps)

        def proj2(dst, wsrc, jp, bias=None, act=None):
            # two 512-col matmuls into one 2-bank PSUM tile.  act=None
            # drains the two banks concurrently on ACT and DVE (halved
            # latency: used pre-attention where the 3-deep PSUM rotation is
            # drain-latency-bound); act=True/False uses one [128, 2, 512]
            # drain on that engine (used when injected into the attention
            # stream, picked opposite to the co-scheduled exp's engine)
            ps = s_pool.tile([128, 2, 512], f32, tag="s")
            for k in range(2):
                nc.tensor.matmul(
                    ps[:, k, :], w_s[wsrc], xkv_t[2 * jp + k],
                    start=True, stop=True,
                )
            if act is None:
                _drain(dst[:, 0:512], ps[:, 0, :], bias, True)
                _drain(dst[:, 512:1024], ps[:, 1, :], bias, False)
            else:
                _drain(dst, ps, bias, act)

        def proj_g(jp, act=None):
            # ghat natural layout [k, m] for key quarter jp, then
            # immediately DMA-transpose that quarter [128, 1024] ->
            # gT [m 128, 8, k 128] on alternating HWDGE queues so PV
            # operands trail the QK stream by as little as possible
            half, sub = jp // 2, jp % 2
            proj2(gn_t[half][:, sub * 1024 : (sub + 1) * 1024], "pg", jp,
                  bias="bg1", act=act)
            # transposes ALWAYS on sync: a DMA_TRANSPOSE issue occupies the
            # issuing engine's queue for ~1.4us, and the scalar (ACT) queue
            # is the contended resource during the fill region
            nc.sync.dma_start_transpose(
                out=gT_t[half][:, sub * 8 : (sub + 1) * 8, :],
                in_=gn_t[half][:, sub * 1024 : (sub + 1) * 1024],
            )

        def proj1(dst, wsrc, jt, bias, act):
            # single 512-col matmul + single [128, 512] drain (~0.6us):
            # the minimum-latency unit for the first QK dependencies
            ps = s_pool.tile([128, 2, 512], f32, tag="s")
            nc.tensor.matmul(ps[:, 0, :], w_s[wsrc], xkv_t[jt],
                             start=True, stop=True)
            _drain(dst, ps[:, 0, :], bias, act)

        # pre-loop: only what QK pairs 0-3 need — theta block-0 queries
        # (cols 0:512) and phi over the first 1024 keys, as 1-MM units
        proj1(theta_s[:, 0:512], "wt", 0, "bt", True)
        proj1(phi_t[0][:, 0:512], "wp", 0, "bp", False)
        proj1(phi_t[0][:, 512:1024], "wp", 1, "bp", True)

        # every other projection is injected into block 0's pair loop
        # (pj -> emit-callback, run right after that pair's QK+exp) so the
        # PE streams matmuls continuously from warmup to the last PV.  The
        # single-engine drain runs opposite to that pair's exp engine.
        def opp(pj):
            return pj in DVE_PAIRS  # exp on DVE -> drain on ACT

        def proj2_fill(dst, wsrc, jp, bias, act):
            # fill-region variant: borrows the (still idle) attn+conv PSUM
            # banks instead of s_pool slots, so the early QK pairs never
            # stall on the 3-deep rotation; BOTH drains go on the engine
            # opposite the co-scheduled exp so neither engine is double-
            # loaded in one pair-step
            pa = attn_pool.tile([128, 512], f32, tag="attn", name="fill_pa")
            pc = conv_pool.tile([128, 512], f32, tag="conv", name="fill_pc")
            nc.tensor.matmul(pa, w_s[wsrc], xkv_t[2 * jp], start=True, stop=True)
            nc.tensor.matmul(pc, w_s[wsrc], xkv_t[2 * jp + 1],
                             start=True, stop=True)
            _drain(dst[:, 0:512], pa, bias, True)
            _drain(dst[:, 512:1024], pc, bias, False)

        def proj_g_fill(jp, act):
            half, sub = jp // 2, jp % 2
            proj2_fill(gn_t[half][:, sub * 1024 : (sub + 1) * 1024], "pg", jp,
                       "bg1", act)
            nc.sync.dma_start_transpose(
                out=gT_t[half][:, sub * 8 : (sub + 1) * 8, :],
                in_=gn_t[half][:, sub * 1024 : (sub + 1) * 1024],
            )

        inject = {
            0: lambda: proj1(theta_s[:, 512:1024], "wt", 1, "bt", opp(0)),
            1: lambda: proj_g_fill(0, opp(1)),
            2: lambda: proj2_fill(phi_t[0][:, 1024:2048], "wp", 1, "bp", opp(2)),
            3: lambda: proj_g(1, act=opp(3)),
            5: lambda: proj2(phi_t[1][:, 0:1024], "wp", 2, "bp", act=opp(5)),
            6: lambda: proj_g(2, act=opp(6)),
            9: lambda: proj2(phi_t[1][:, 1024:2048], "wp", 3, "bp", act=opp(9)),
            11: lambda: proj_g(3, act=opp(11)),
            12: lambda: proj2(theta_s[:, 1024:2048], "wt", 1, "bt", act=opp(12)),
        }

        # ---- attention ----

        pending = None  # (attn_ps, q0, blk) of the previous block

        def finish_block(attn_ps, q0, blk, last=False):
            if not last:
                yu = small.tile([128, 512], bf16, tag="yu")
                nc.vector.tensor_copy(out=yu, in_=attn_ps)
                conv_ps = conv_pool.tile([128, 512], f32, tag="conv")
                nc.tensor.matmul(conv_ps, w_s["co"], yu, start=True, stop=True)
                out_s = small.tile([128, 512], bf16, tag="out_s")
                nc.vector.tensor_copy(out=out_s, in_=conv_ps)
                nc.sync.dma_start(out=out_d[:, q0 : q0 + NBLK], in_=out_s)
                # den issue on sync too: a mid-stream DMA issue on the
                # scalar queue delays an exp by up to ~550ns once per block
                nc.sync.dma_start(out=den_d[blk : blk + 1, :], in_=yu[0:1, :])
            else:
                # tail: two half chunks; each half's cast/conv/copy chain
                # ping-pongs ACT<->DVE and its out-DMA issues as soon as its
                # copy lands, so the first 64KB transfer overlaps the second
                # half's compute.  One den DMA reads row 0 of the whole yu.
                conv_ps = conv_pool.tile([128, 512], f32, tag="conv")
                yu = small.tile([128, 512], bf16, tag="yu")
                out_s = small.tile([128, 512], bf16, tag="out_s")
                # both casts FIRST (ACT half + DVE half in parallel) so
                # neither conv waits behind an out_s copy on its engine
                nc.scalar.activation(
                    out=yu[:, 0:256], in_=attn_ps[:, 0:256], func=Copy,
                    bias=0.0, scale=1.0,
                )
                nc.vector.tensor_copy(out=yu[:, 256:512], in_=attn_ps[:, 256:512])
                for hh in range(2):
                    sl = slice(hh * 256, (hh + 1) * 256)
                    nc.tensor.matmul(
                        conv_ps[:, sl], w_s["co"], yu[:, sl],
                        start=True, stop=True,
                    )
                    if hh == 0:
                        nc.vector.tensor_copy(out=out_s[:, sl], in_=conv_ps[:, sl])
                    else:
                        nc.scalar.activation(
                            out=out_s[:, sl], in_=conv_ps[:, sl], func=Copy,
                            bias=0.0, scale=1.0,
                        )
                    nc.sync.dma_start(
                        out=out_d[:, q0 + hh * 256 : q0 + (hh + 1) * 256],
                        in_=out_s[:, sl],
                    )
                # den on sync AFTER the out chunks: on scalar its ~0.6us
                # issue would sit between the ACT cast and the final out_s
                # copy, delaying the chain end
                nc.sync.dma_start(out=den_d[blk : blk + 1, :], in_=yu[0:1, :])

        # ---- flat pair stream: one software pipeline over all 64 pairs of
        # all 4 blocks.  QK(pg) runs PVD steps ahead of PV(pg-PVD); the PV
        # stream crosses block boundaries without draining, so the PE never
        # bunches up on the exp engines at block edges. ----
        NPAIR = NMCH // 2
        NB = QH // NBLK
        TOT = NB * NPAIR
        pt_tiles = []
        attn_tiles = {}
        next_pv = 0
        pending = None
        for pg in range(TOT + PVD):
            blk, pj = divmod(pg, NPAIR)
            if pg < TOT:
                thq = theta_s[:, blk * NBLK : (blk + 1) * NBLK]
                sp = s_pool.tile([128, 2, 512], f32, tag="s")
                for k2 in range(2):
                    mi = pj * 2 + k2
                    nc.tensor.matmul(
                        sp[:, k2, :],
                        phi_t[mi // 16][:, (mi % 16) * 128 : (mi % 16 + 1) * 128],
                        thq,
                        start=True,
                        stop=True,
                    )
                pt = pt_pool.tile([128, 2, 512], bf16, tag="pt")
                if pg >= TOT - 6:
                    # final pairs: split each exp across both engines so the
                    # last PVs and the tail epilogue start ~600ns earlier
                    nc.scalar.activation(
                        out=pt[:, 0, :], in_=sp[:, 0, :], func=Exp,
                        bias=0.0, scale=1.0,
                    )
                    nc.vector.tensor_scalar(
                        out=pt[:, 1, :].bitcast(i16),
                        in0=sp[:, 1, :],
                        scalar1=EXP_A16,
                        scalar2=EXP_B16,
                        op0=mybir.AluOpType.mult,
                        op1=mybir.AluOpType.add,
                    )
                elif pj in DVE_PAIRS:
                    # Schraudolph exp on DVE: int16(A*s+B) bits == bf16 P
                    nc.vector.tensor_scalar(
                        out=pt.bitcast(i16),
                        in0=sp,
                        scalar1=EXP_A16,
                        scalar2=EXP_B16,
                        op0=mybir.AluOpType.mult,
                        op1=mybir.AluOpType.add,
                    )
                else:
                    nc.scalar.activation(
                        out=pt, in_=sp, func=Exp, bias=0.0, scale=1.0
                    )
                pt_tiles.append(pt)
                if pg in inject:
                    inject[pg]()
            # PV drain: trails by PVD; block 0 additionally gated until
            # pg==6 so the ghat DMA-transposes land first
            while next_pv <= pg - PVD and next_pv < TOT:
                if next_pv < NPAIR and pg < 5:
                    break
                p = next_pv
                next_pv += 1
                b2, pj2 = divmod(p, NPAIR)
                if pj2 == 0:
                    attn_tiles[b2] = attn_pool.tile(
                        [128, 512], f32, tag="attn", name=f"attn{b2}"
                    )
                for k2 in range(2):
                    mi = pj2 * 2 + k2
                    nc.tensor.matmul(
                        attn_tiles[b2],
                        gT_t[mi // 16][:, mi % 16, :],
                        pt_tiles[p][:, k2, :],
                        start=(mi == 0),
                        stop=(mi == NMCH - 1),
                    )
                if pj2 == NPAIR - 1:
                    pending = (attn_tiles[b2], b2 * NBLK, b2)
                    pt_tiles[p - NPAIR + 1 : p + 1] = [None] * NPAIR
            if pending is not None and pending[2] < NB - 1:
                finish_block(*pending)
                pending = None
        finish_block(*pending, last=True)

    # populate .instr bytes for extended-inst InstISA subclasses — raw Bass
    # skips this pass and the NEFF compiler fails "ISA wrong length"
    mybir.codegen_inst_isa_subclasses(nc)

    import json as _json
    import os as _os

    blob = _json.dumps(
        _legalize_waits(
            _json.loads(nc.to_json_bytes()),
            verbose=bool(_os.environ.get("KERNEL_DEBUG")),
        )
    ).encode()
    nc.to_json_bytes = lambda: blob
    return nc


def _get_nc():
    if "nc" not in _CACHE:
        _CACHE["nc"] = _build()
    return _CACHE["nc"]


def _prep_host(inputs):
    """Host-side precompute: weight transposes, fused G = w_out@w_g SVD
    split (rank 127 + ones/denominator channel at k=0), fused bias, and
    bf16 casts + packing of all device weight inputs."""
    import ml_dtypes

    bf16 = ml_dtypes.bfloat16
    w_g = np.asarray(inputs["w_g"], np.float32)
    w_out = np.asarray(inputs["w_out"], np.float32)
    G = w_out @ w_g
    U, S, Vt = np.linalg.svd(G)
    r = 127
    pg = np.zeros((C, C), np.float32)  # lhsT: pg[c, k] = P_g[k-1, c]
    pg[:, 1 : r + 1] = Vt[:r, :].T
    co = np.zeros((C, C), np.float32)  # lhsT: co[k, c] = C_out[c, k-1]
    co[1 : r + 1, :] = (U[:, :r] * S[:r][None, :]).T
    bcomb = (
        np.asarray(inputs["b_out"], np.float32)
        + w_out @ np.asarray(inputs["b_g"], np.float32)
    ).reshape(C, 1)
    wts = np.concatenate(
        [
            np.asarray(inputs["w_theta"], np.float32).T,
            np.asarray(inputs["w_phi"], np.float32).T,
            pg,
            co,
        ],
        axis=1,
    ).astype(bf16)
    bg1 = np.zeros((C, 1), np.float32)
    bg1[0, 0] = 1.0  # ones channel: ghat row 0 = 0 + 1 via the drain bias
    bias = np.concatenate(
        [
            np.asarray(inputs["b_theta"], np.float32).reshape(C, 1),
            np.asarray(inputs["b_phi"], np.float32).reshape(C, 1),
            bg1,
        ],
        axis=1,
    )
    wmaps = {
        "wts": np.ascontiguousarray(wts),
        "bias": np.ascontiguousarray(bias),
    }
    return wmaps, bcomb


def _run(inputs, trace=False, **spmd_kwargs):
    import ml_dtypes

    from concourse.bass_utils import run_bass_kernel_spmd

    x = np.asarray(inputs["x"], np.float32)
    xf = np.ascontiguousarray(x.reshape(B, C, HW))
    wmaps, bcomb = _prep_host(inputs)
    in_maps = []
    for k in range(NCORES):
        b, h = k // 2, k % 2
        # rotate keys so this core's queries are columns [0, QH)
        xkv = np.ascontiguousarray(
            np.roll(xf[b], -h * QH, axis=1).astype(ml_dtypes.bfloat16)
        )
        in_maps.append({"xkv": xkv, **wmaps})
    nc = _get_nc()
    res = run_bass_kernel_spmd(
        nc, in_maps, core_ids=list(range(NCORES)), trace=trace, **spmd_kwargs
    )
    out = np.empty((B, C, HW), np.float32)
    for k in range(NCORES):
        b, h = k // 2, k % 2
        conv_u = np.asarray(  # [C, QH], unnormalized conv result (bf16)
            res.results[k]["out"], np.float32
        )
        den = np.asarray(  # softmax denominators (bf16 row 0 of yu)
            res.results[k]["den"], np.float32
        ).reshape(QH)
        xq = xf[b][:, h * QH : (h + 1) * QH]
        out[b][:, h * QH : (h + 1) * QH] = conv_u / den[None, :] + xq + bcomb
    return out.reshape(B, C, 64, 64), res


def kernel(**inputs):
    out, _ = _run(inputs, trace=False)
    return out

